# revision 6
# baseline (speedup 1.0000x reference)
"""Trainium2 Bass kernel for the RadialBasisArbitraryLayerT problem.

Math: for each pixel p=(y,x) and batch b:
    flow[b,ch,p] = sum_m phi[b,p,m] * alpha[b, idx[p,m], ch]
    phi[b,p,m]   = phi_0[p,m] + (lx[b,i]-cx0[p,m])*phi_x[p,m]
                              + (ly[b,i]-cy0[p,m])*phi_y[p,m],  i = idx[p,m]

All neighbor indices of a pixel live in a 6x6 window of the 32x32 control
grid, so the constant-index gather is converted into small dense matmuls:

    flow[bc, p] = sum_k  W[p, k] * R[k, bc]        k = (dx, dy, c) = 6*7*3 = 126
    W[p,(dx,dy,0)] = phi_0 - cx0*phi_x - cy0*phi_y   (constant, host-built)
    W[p,(dx,dy,1)] = phi_x,  W[p,(dx,dy,2)] = phi_y
    R[(dx,dy,c), (b,ch)] = l_c[b,i] * alpha[b,i,ch], l = (1, lx, ly)

W is a compile-time constant streamed from DRAM; R is computed on-chip from
a tiny host-gathered rearrangement (m1*m2) of the runtime inputs.

Sharding: 32 image rows (4 y-tiles of 8 rows) per core, 8 cores.
Each y-tile x x-band is one matmul: lhsT=R_chunk [126,16], rhs=W_tile [126,N],
out=PSUM[16, N] packed 4 stripes high (tile_position col groups) so PSUM
eviction runs 128 partitions wide. Output staged in SBUF, DMA'd per stripe
lane, final (y,x) reassembly on host.
"""

import numpy as np

import concourse.bass as bass
import concourse.tile as tile
from concourse import mybir
from concourse.bass_utils import run_bass_kernel_spmd

# ---------------------------------------------------------------- geometry
H = 256
W_IMG = 256
CH = 32
CW = 32
C = 24.0
B = 8
K = CH * CW
NCORES = 8
YT = 8                 # rows per y-tile
NYT = H // YT          # 32 y-tiles
YTPC = NYT // NCORES   # 4 y-tiles per core
KDIM = 6 * 7 * 3       # (dx, dy, c) window rows = 126
BC = B * 2             # 16 output channels (b, ch)
PSUM_COLS = 512

DT = mybir.dt.float32          # on-chip dtype for W / R
NPDT = np.float32

_cache = {}


def _legalize_sync_waits(nc, max_waits=1):
    """This walrus build rejects instructions with more than one sync wait.
    Hoist extra waits into single-wait NOPs inserted just before the
    instruction on the same (sequential) engine — semantics preserved."""
    import bass_rust

    uid = 0
    for fn in nc.m.functions:
        for blk in fn.blocks:
            insts = blk.instructions
            new = []
            changed = False
            for inst in insts:
                si = inst.sync_info
                if si is not None and len(si.on_wait) > max_waits:
                    waits = list(si.on_wait)
                    for w in waits[:-max_waits]:
                        nop = bass_rust.InstNoOp(
                            name=f"legal-wait-{uid}",
                            engine=inst.engine,
                            ins=[],
                            outs=[],
                            sync_info=bass_rust.SyncInfo(on_wait=[w], on_update=[]),
                        )
                        uid += 1
                        new.append(nop)
                    inst.sync_info = bass_rust.SyncInfo(
                        on_wait=waits[-max_waits:], on_update=si.on_update
                    )
                    changed = True
                new.append(inst)
            if changed:
                blk.instructions = new


def _build_buffers():
    """Recompute the constant neighbor structure (mirrors the reference)."""
    cy = np.linspace(0.0, H - 1, CH, dtype=np.float32)
    cx = np.linspace(0.0, W_IMG - 1, CW, dtype=np.float32)
    gy, gx = np.meshgrid(cy, cx, indexing="ij")
    cp = np.stack([gx, gy], axis=-1).reshape(-1, 2).astype(np.float32)
    iy, ix = np.meshgrid(
        np.arange(H, dtype=np.float32),
        np.arange(W_IMG, dtype=np.float32),
        indexing="ij",
    )
    img = np.stack([ix, iy], axis=-1)
    dist = (
        np.linalg.norm(img[:, :, None, :] - cp[None, None, :, :], axis=3).astype(
            np.float32
        )
        / np.float32(C)
    )
    idx = np.argsort(dist, axis=2, kind="stable")
    sd = np.take_along_axis(dist, idx, axis=2)
    M = int((dist < 1.0).sum(axis=2).max())
    sd = sd[..., :M]
    idx = idx[..., :M].astype(np.int32)
    mask = (sd < 1.0).astype(np.float32)
    scp = cp[idx]
    one_m = 1.0 - sd
    phi_0 = (one_m**4) * (4.0 * sd + 1.0) * mask
    phi_r = -4.0 * (one_m**3) * (4.0 * sd + 1.0) + 4.0 * (one_m**4)
    denom = sd * np.float32(C * C) + np.float32(1e-5)
    r_x = (scp[..., 0] - img[:, :, None, 0]) / denom
    r_y = (scp[..., 1] - img[:, :, None, 1]) / denom
    phi_x = (phi_r * r_x * mask).astype(np.float32)
    phi_y = (phi_r * r_y * mask).astype(np.float32)
    return idx, phi_0.astype(np.float32), phi_x, phi_y, scp.astype(np.float32), mask


def _geometry():
    """Static tiling metadata: x-bands, y-windows, stripe packing, index maps."""
    if "geom" in _cache:
        return _cache["geom"]

    idx, phi_0, phi_x, phi_y, scp, mask = _build_buffers()
    M = idx.shape[-1]
    bmask = mask > 0.5
    gyi = idx // CW
    gxi = idx % CW

    # x-bands: maximal runs of x sharing one 6-wide gx window
    gx_min = np.where(bmask, gxi, 999).min(axis=(0, 2))
    gx_max = np.where(bmask, gxi, -1).max(axis=(0, 2))
    gx0 = np.minimum(gx_min, CW - 6)
    assert (gx_max - gx0 <= 5).all() and (gx0 >= 0).all()
    bands = []  # (x_start, width, gx0)
    s = 0
    for x in range(1, W_IMG + 1):
        if x == W_IMG or gx0[x] != gx0[s]:
            bands.append((s, x - s, int(gx0[s])))
            s = x
    NB = len(bands)

    # y-tiles: 8 rows, 7-wide gy window
    gy_min_row = np.where(bmask, gyi, 999).min(axis=(1, 2))
    gy_max_row = np.where(bmask, gyi, -1).max(axis=(1, 2))
    sy7 = []
    for t in range(NYT):
        lo = int(gy_min_row[YT * t : YT * t + YT].min())
        hi = int(gy_max_row[YT * t : YT * t + YT].max())
        sy = min(lo, CH - 7)
        assert hi - sy <= 6 and sy >= 0
        sy7.append(sy)

    # matmul N per band and greedy PSUM stripe packing (identical per y-tile)
    Ns = [YT * w for (_, w, _) in bands]
    stripe_of = []
    col_of = []
    sid, cur = 0, 0
    for N in Ns:
        if cur + N > PSUM_COLS:
            sid += 1
            cur = 0
        stripe_of.append(sid)
        col_of.append(cur)
        cur += N
    SPT = sid + 1                       # stripes per y-tile
    NSTRIPE = YTPC * SPT                # global stripes per core
    NGROUP = (NSTRIPE + 3) // 4         # PSUM tiles (4 stripes each) per core

    # per-chunk control point index map: I[ty, band, k] (k = dx*21 + dy*3 + c)
    dxr = np.arange(KDIM) // 21
    dy = (np.arange(KDIM) % 21) // 3
    c_of_k = np.arange(KDIM) % 3
    I_map = np.empty((NYT, NB, KDIM), np.int64)
    for t in range(NYT):
        for bi, (_, _, g0) in enumerate(bands):
            I_map[t, bi] = (sy7[t] + dy) * CW + (g0 + dxr)
    assert I_map.min() >= 0 and I_map.max() < K

    geom = dict(
        bands=bands, NB=NB, sy7=sy7, Ns=Ns, stripe_of=stripe_of, col_of=col_of,
        SPT=SPT, NSTRIPE=NSTRIPE, NGROUP=NGROUP, I_map=I_map, c_of_k=c_of_k,
        idx=idx, mask=bmask, phi_0=phi_0, phi_x=phi_x, phi_y=phi_y, scp=scp, M=M,
    )
    _cache["geom"] = geom
    return geom


def _build_w():
    """Constant weights W[ytile, KDIM, 2048] from the phi buffers."""
    if "w" in _cache:
        return _cache["w"]
    g = _geometry()
    bands, sy7 = g["bands"], g["sy7"]
    idx, bmask = g["idx"], g["mask"]
    phi_x, phi_y = g["phi_x"], g["phi_y"]
    a0 = g["phi_0"] - g["scp"][..., 0] * phi_x - g["scp"][..., 1] * phi_y
    M = g["M"]

    band_of_x = np.empty(W_IMG, np.int64)
    xs_of_x = np.empty(W_IMG, np.int64)
    wd_of_x = np.empty(W_IMG, np.int64)
    for bi, (xs, wd, _) in enumerate(bands):
        band_of_x[xs : xs + wd] = bi
        xs_of_x[xs : xs + wd] = xs
        wd_of_x[xs : xs + wd] = wd
    g0_of_x = np.array([bands[bi][2] for bi in band_of_x])

    yy, xx, _mm = np.meshgrid(
        np.arange(H), np.arange(W_IMG), np.arange(M), indexing="ij"
    )
    ty = yy // YT
    yl = yy % YT
    col = xs_of_x[xx] * YT + yl * wd_of_x[xx] + (xx - xs_of_x[xx])
    gyi = idx // CW
    gxi = idx % CW
    dy = gyi - np.array(sy7)[ty]
    dxr = gxi - g0_of_x[xx]
    v = bmask
    assert (dy[v] >= 0).all() and (dy[v] <= 6).all()
    assert (dxr[v] >= 0).all() and (dxr[v] <= 5).all()
    k = dxr * 21 + dy * 3

    w = np.zeros((NYT, KDIM, YT * W_IMG), np.float32)
    flat = (ty * KDIM + k) * (YT * W_IMG) + col
    for cc, vals in enumerate((a0, phi_x, phi_y)):
        np.add.at(w.ravel(), flat[v] + cc * (YT * W_IMG), vals[v])
    _cache["w"] = w
    return w


def _build_program():
    """Build the SPMD Bass program (same for every core)."""
    if "nc" in _cache:
        return _cache["nc"]
    g = _geometry()
    NB, Ns = g["NB"], g["Ns"]
    stripe_of, col_of, SPT, NGROUP = g["stripe_of"], g["col_of"], g["SPT"], g["NGROUP"]
    COLS = YT * W_IMG  # 2048 W columns per y-tile
    RCOLS = YTPC * NB * BC  # 1728 R columns

    nc = bass.Bass()
    w_p = nc.declare_dram_parameter("w", [YTPC, KDIM, COLS], DT, isOutput=False)
    m1_p = nc.declare_dram_parameter("m1", [KDIM, RCOLS], DT, isOutput=False)
    m2_p = nc.declare_dram_parameter("m2", [KDIM, RCOLS], DT, isOutput=False)
    out_p = nc.declare_dram_parameter(
        "out", [4, BC, NGROUP * PSUM_COLS], mybir.dt.float32, isOutput=True
    )

    with tile.TileContext(nc) as tc:
        with (
            tc.tile_pool(name="const", bufs=1) as const,
            tc.tile_pool(name="psum", bufs=4, space="PSUM") as psum,
            tc.tile_pool(name="wpsum", bufs=1, space="PSUM") as wpsum,
            tc.tile_pool(name="stage", bufs=1) as stagep,
        ):
            # W resident in SBUF; per-y-tile DMAs spread over both HWDGE rings
            w_ts = []
            for ty in range(YTPC):
                w_t = const.tile([KDIM, COLS], DT, name=f"w{ty}", tag=f"w{ty}")
                import os as _os
                if _os.environ.get("K_RINGS", "1") == "1":
                    eng = nc.sync if ty % 2 == 0 else nc.scalar
                else:
                    eng = nc.sync
                eng.dma_start(out=w_t[:], in_=w_p[ty])
                w_ts.append(w_t)

            # runtime tables on the SWDGE ring, concurrent with W
            m1_t = const.tile([KDIM, RCOLS], DT)
            m2_t = const.tile([KDIM, RCOLS], DT)
            r_t = const.tile([KDIM, RCOLS], DT)
            import os as _os
            _m_eng = nc.gpsimd if _os.environ.get("K_RINGS", "1") == "1" else nc.sync
            _m_eng.dma_start(out=m1_t[:], in_=m1_p[:])
            _m_eng.dma_start(out=m2_t[:], in_=m2_p[:])

            # PE warmup while DMAs land: junk matmuls into a scratch bank
            # (~5us of PE activity flips HAM to 8/8 before the real phase)
            import os as _os
            if _os.environ.get("K_WARMUP", "1") == "1":
                wu_a = const.tile([KDIM, 16], DT)
                wu_b = const.tile([KDIM, 128], DT)
                nc.vector.memset(wu_a[:], 0.0)
                nc.vector.memset(wu_b[:], 0.0)
                wu_p = wpsum.tile([128, 128], mybir.dt.float32)
                for _ in range(48):
                    nc.tensor.matmul(
                        wu_p[:16, :], wu_a[:], wu_b[:], start=True, stop=True,
                        tile_position=(0, 0),
                    )

            nc.vector.tensor_mul(r_t[:], m1_t[:], m2_t[:])

            stage_t = stagep.tile([128, NGROUP * PSUM_COLS], mybir.dt.float32)

            cur_group = -1
            pt = None
            for ty in range(YTPC):
                w_t = w_ts[ty]
                for bi in range(NB):
                    gs = ty * SPT + stripe_of[bi]  # global stripe
                    grp, lane = gs // 4, gs % 4
                    if grp != cur_group:
                        if pt is not None:
                            nc.any.tensor_copy(
                                stage_t[:, cur_group * PSUM_COLS:(cur_group + 1) * PSUM_COLS],
                                pt[:],
                            )
                        pt = psum.tile([128, PSUM_COLS], mybir.dt.float32, tag="pt")
                        cur_group = grp
                    xs = g["bands"][bi][0]
                    rcol = (ty * NB + bi) * BC
                    nc.tensor.matmul(
                        pt[32 * lane : 32 * lane + BC, col_of[bi] : col_of[bi] + Ns[bi]],
                        r_t[:, rcol : rcol + BC],
                        w_t[:, YT * xs : YT * xs + Ns[bi]],
                        start=True,
                        stop=True,
                        tile_position=(0, 32 * lane),
                    )
            nc.any.tensor_copy(
                stage_t[:, cur_group * PSUM_COLS:(cur_group + 1) * PSUM_COLS], pt[:]
            )
            for lane in range(4):
                eng = nc.sync if (lane % 2 == 0 or _os.environ.get("K_RINGS", "1") != "1") else nc.scalar
                eng.dma_start(
                    out=out_p[lane], in_=stage_t[32 * lane : 32 * lane + BC, :]
                )

    _legalize_sync_waits(nc)
    _cache["nc"] = nc
    return nc


def _host_inputs(cpoint_loc, alpha):
    """Per-core m1/m2 rearrangements of the runtime inputs."""
    g = _geometry()
    I_map, c_of_k, NB = g["I_map"], g["c_of_k"], g["NB"]
    lx = cpoint_loc[..., 0]
    ly = cpoint_loc[..., 1]
    lval = np.stack([np.ones_like(lx), lx, ly])  # [3, B, K]

    # m1[ty, band, k, b] = lval[c(k), b, I[ty,band,k]] ; duplicated over ch
    m1b = lval[
        c_of_k[None, None, :, None],
        np.arange(B)[None, None, None, :],
        I_map[:, :, :, None],
    ]  # [NYT, NB, KDIM, B]
    m1 = np.repeat(m1b, 2, axis=3)  # [NYT, NB, KDIM, 16]
    m2 = alpha[
        np.arange(B)[None, None, None, :, None],
        I_map[:, :, :, None, None],
        np.arange(2)[None, None, None, None, :],
    ].reshape(NYT, NB, KDIM, BC)

    in_maps = []
    w = _build_w()
    npdt = mybir.dt.np(DT)
    for core in range(NCORES):
        sl = slice(core * YTPC, (core + 1) * YTPC)
        m1c = m1[sl].transpose(2, 0, 1, 3).reshape(KDIM, -1)
        m2c = m2[sl].transpose(2, 0, 1, 3).reshape(KDIM, -1)
        in_maps.append(
            {
                "w": np.ascontiguousarray(w[sl]).astype(npdt),
                "m1": np.ascontiguousarray(m1c).astype(npdt),
                "m2": np.ascontiguousarray(m2c).astype(npdt),
            }
        )
    return in_maps


def _unshuffle(results):
    """Reassemble [B, 2, H, W] from the per-core staged outputs."""
    g = _geometry()
    bands, Ns = g["bands"], g["Ns"]
    stripe_of, col_of, SPT = g["stripe_of"], g["col_of"], g["SPT"]
    out = np.empty((B, 2, H, W_IMG), np.float32)
    for core in range(NCORES):
        res = results[core]["out"]  # [4, BC, NGROUP*512]
        for ty in range(YTPC):
            y0 = (core * YTPC + ty) * YT
            for bi, (xs, wd, _) in enumerate(bands):
                gs = ty * SPT + stripe_of[bi]
                grp, lane = gs // 4, gs % 4
                block = res[lane][:, grp * PSUM_COLS + col_of[bi] :
                                  grp * PSUM_COLS + col_of[bi] + Ns[bi]]
                out[:, :, y0 : y0 + YT, xs : xs + wd] = block.reshape(
                    B, 2, YT, wd
                )
    return out


def run(cpoint_loc, alpha, trace=False, trace_cores=None):
    nc = _build_program()
    in_maps = _host_inputs(np.asarray(cpoint_loc), np.asarray(alpha))
    res = run_bass_kernel_spmd(
        nc, in_maps, list(range(NCORES)), trace=trace,
        trace_cores=trace_cores if trace_cores is not None else
        (list(range(NCORES)) if trace else None),
    )
    return _unshuffle(res.results), res


def kernel(cpoint_loc, alpha, select_index=None, phi_0=None, phi_x=None,
           phi_y=None, cpoints_0=None, **_ignored):
    out, _ = run(np.asarray(cpoint_loc), np.asarray(alpha))
    return out


# revision 8
# speedup vs baseline: 1.8441x; 1.8441x over previous
"""Trainium2 Bass kernel for the RadialBasisArbitraryLayerT problem.

Math: for each pixel p=(y,x) and batch b:
    flow[b,ch,p] = sum_m phi[b,p,m] * alpha[b, idx[p,m], ch]
    phi[b,p,m]   = phi_0[p,m] + (lx[b,i]-cx0[p,m])*phi_x[p,m]
                              + (ly[b,i]-cy0[p,m])*phi_y[p,m],  i = idx[p,m]

All neighbor indices of a pixel live in a 6x6 window of the 32x32 control
grid, so the constant-index gather is converted into small dense matmuls:

    flow[bc, p] = sum_k  W[p, k] * R[k, bc]        k = (dx, dy, c) = 6*7*3 = 126
    W[p,(dx,dy,0)] = phi_0 - cx0*phi_x - cy0*phi_y   (constant, host-built)
    W[p,(dx,dy,1)] = phi_x,  W[p,(dx,dy,2)] = phi_y
    R[(dx,dy,c), (b,ch)] = l_c[b,i] * alpha[b,i,ch], l = (1, lx, ly)

W is a compile-time constant streamed from DRAM; R is computed on-chip from
a tiny host-gathered rearrangement (m1*m2) of the runtime inputs.

Sharding: 32 image rows (4 y-tiles of 8 rows) per core, 8 cores.
Each y-tile x x-band is one matmul: lhsT=R_chunk [126,16], rhs=W_tile [126,N],
out=PSUM[16, N] packed 4 stripes high (tile_position col groups) so PSUM
eviction runs 128 partitions wide. Output staged in SBUF, DMA'd per stripe
lane, final (y,x) reassembly on host.
"""

import numpy as np

import concourse.bass as bass
import concourse.tile as tile
from concourse import mybir
from concourse.bass_utils import run_bass_kernel_spmd

# ---------------------------------------------------------------- geometry
H = 256
W_IMG = 256
CH = 32
CW = 32
C = 24.0
B = 8
K = CH * CW
NCORES = 8
YT = 8                 # rows per y-tile
NYT = H // YT          # 32 y-tiles
YTPC = NYT // NCORES   # 4 y-tiles per core
KDIM = 6 * 7 * 3       # (dx, dy, c) window rows = 126
KP = 128               # padded contraction dim (DMA ports + FWL want 128)
BC = B * 2             # 16 output channels (b, ch)
PSUM_COLS = 512

DT = mybir.dt.float16          # on-chip dtype for W / R
NPDT = np.float16

_cache = {}


def _legalize_sync_waits(nc, max_waits=1):
    """This walrus build rejects instructions with more than one sync wait.
    Hoist extra waits into single-wait NOPs inserted just before the
    instruction on the same (sequential) engine — semantics preserved."""
    import bass_rust

    uid = 0
    for fn in nc.m.functions:
        for blk in fn.blocks:
            insts = blk.instructions
            new = []
            changed = False
            for inst in insts:
                si = inst.sync_info
                if si is not None and len(si.on_wait) > max_waits:
                    waits = list(si.on_wait)
                    for w in waits[:-max_waits]:
                        nop = bass_rust.InstNoOp(
                            name=f"legal-wait-{uid}",
                            engine=inst.engine,
                            ins=[],
                            outs=[],
                            sync_info=bass_rust.SyncInfo(on_wait=[w], on_update=[]),
                        )
                        uid += 1
                        new.append(nop)
                    inst.sync_info = bass_rust.SyncInfo(
                        on_wait=waits[-max_waits:], on_update=si.on_update
                    )
                    changed = True
                new.append(inst)
            if changed:
                blk.instructions = new


def _build_buffers():
    """Recompute the constant neighbor structure (mirrors the reference)."""
    cy = np.linspace(0.0, H - 1, CH, dtype=np.float32)
    cx = np.linspace(0.0, W_IMG - 1, CW, dtype=np.float32)
    gy, gx = np.meshgrid(cy, cx, indexing="ij")
    cp = np.stack([gx, gy], axis=-1).reshape(-1, 2).astype(np.float32)
    iy, ix = np.meshgrid(
        np.arange(H, dtype=np.float32),
        np.arange(W_IMG, dtype=np.float32),
        indexing="ij",
    )
    img = np.stack([ix, iy], axis=-1)
    dist = (
        np.linalg.norm(img[:, :, None, :] - cp[None, None, :, :], axis=3).astype(
            np.float32
        )
        / np.float32(C)
    )
    idx = np.argsort(dist, axis=2, kind="stable")
    sd = np.take_along_axis(dist, idx, axis=2)
    M = int((dist < 1.0).sum(axis=2).max())
    sd = sd[..., :M]
    idx = idx[..., :M].astype(np.int32)
    mask = (sd < 1.0).astype(np.float32)
    scp = cp[idx]
    one_m = 1.0 - sd
    phi_0 = (one_m**4) * (4.0 * sd + 1.0) * mask
    phi_r = -4.0 * (one_m**3) * (4.0 * sd + 1.0) + 4.0 * (one_m**4)
    denom = sd * np.float32(C * C) + np.float32(1e-5)
    r_x = (scp[..., 0] - img[:, :, None, 0]) / denom
    r_y = (scp[..., 1] - img[:, :, None, 1]) / denom
    phi_x = (phi_r * r_x * mask).astype(np.float32)
    phi_y = (phi_r * r_y * mask).astype(np.float32)
    return idx, phi_0.astype(np.float32), phi_x, phi_y, scp.astype(np.float32), mask


def _geometry():
    """Static tiling metadata: x-bands, y-windows, stripe packing, index maps."""
    if "geom" in _cache:
        return _cache["geom"]

    idx, phi_0, phi_x, phi_y, scp, mask = _build_buffers()
    M = idx.shape[-1]
    bmask = mask > 0.5
    gyi = idx // CW
    gxi = idx % CW

    # x-bands: maximal runs of x sharing one 6-wide gx window
    gx_min = np.where(bmask, gxi, 999).min(axis=(0, 2))
    gx_max = np.where(bmask, gxi, -1).max(axis=(0, 2))
    gx0 = np.minimum(gx_min, CW - 6)
    assert (gx_max - gx0 <= 5).all() and (gx0 >= 0).all()
    bands = []  # (x_start, width, gx0)
    s = 0
    for x in range(1, W_IMG + 1):
        if x == W_IMG or gx0[x] != gx0[s]:
            bands.append((s, x - s, int(gx0[s])))
            s = x
    NB = len(bands)

    # y-tiles: 8 rows, 7-wide gy window
    gy_min_row = np.where(bmask, gyi, 999).min(axis=(1, 2))
    gy_max_row = np.where(bmask, gyi, -1).max(axis=(1, 2))
    sy7 = []
    for t in range(NYT):
        lo = int(gy_min_row[YT * t : YT * t + YT].min())
        hi = int(gy_max_row[YT * t : YT * t + YT].max())
        sy = min(lo, CH - 7)
        assert hi - sy <= 6 and sy >= 0
        sy7.append(sy)

    # matmul N per band and greedy PSUM stripe packing (identical per y-tile)
    Ns = [YT * w for (_, w, _) in bands]
    stripe_of = []
    col_of = []
    sid, cur = 0, 0
    for N in Ns:
        if cur + N > PSUM_COLS:
            sid += 1
            cur = 0
        stripe_of.append(sid)
        col_of.append(cur)
        cur += N
    SPT = sid + 1                       # stripes per y-tile
    NSTRIPE = YTPC * SPT                # global stripes per core
    NGROUP = (NSTRIPE + 3) // 4         # PSUM tiles (4 stripes each) per core

    # per-chunk control point index map: I[ty, band, k] (k = dx*21 + dy*3 + c)
    dxr = np.arange(KDIM) // 21
    dy = (np.arange(KDIM) % 21) // 3
    c_of_k = np.arange(KDIM) % 3
    I_map = np.empty((NYT, NB, KDIM), np.int64)
    for t in range(NYT):
        for bi, (_, _, g0) in enumerate(bands):
            I_map[t, bi] = (sy7[t] + dy) * CW + (g0 + dxr)
    assert I_map.min() >= 0 and I_map.max() < K

    geom = dict(
        bands=bands, NB=NB, sy7=sy7, Ns=Ns, stripe_of=stripe_of, col_of=col_of,
        SPT=SPT, NSTRIPE=NSTRIPE, NGROUP=NGROUP, I_map=I_map, c_of_k=c_of_k,
        idx=idx, mask=bmask, phi_0=phi_0, phi_x=phi_x, phi_y=phi_y, scp=scp, M=M,
    )
    _cache["geom"] = geom
    return geom


def _build_w():
    """Constant weights W[ytile, KDIM, 2048] from the phi buffers."""
    if "w" in _cache:
        return _cache["w"]
    g = _geometry()
    bands, sy7 = g["bands"], g["sy7"]
    idx, bmask = g["idx"], g["mask"]
    phi_x, phi_y = g["phi_x"], g["phi_y"]
    a0 = g["phi_0"] - g["scp"][..., 0] * phi_x - g["scp"][..., 1] * phi_y
    M = g["M"]

    band_of_x = np.empty(W_IMG, np.int64)
    xs_of_x = np.empty(W_IMG, np.int64)
    wd_of_x = np.empty(W_IMG, np.int64)
    for bi, (xs, wd, _) in enumerate(bands):
        band_of_x[xs : xs + wd] = bi
        xs_of_x[xs : xs + wd] = xs
        wd_of_x[xs : xs + wd] = wd
    g0_of_x = np.array([bands[bi][2] for bi in band_of_x])

    yy, xx, _mm = np.meshgrid(
        np.arange(H), np.arange(W_IMG), np.arange(M), indexing="ij"
    )
    ty = yy // YT
    yl = yy % YT
    col = xs_of_x[xx] * YT + yl * wd_of_x[xx] + (xx - xs_of_x[xx])
    gyi = idx // CW
    gxi = idx % CW
    dy = gyi - np.array(sy7)[ty]
    dxr = gxi - g0_of_x[xx]
    v = bmask
    assert (dy[v] >= 0).all() and (dy[v] <= 6).all()
    assert (dxr[v] >= 0).all() and (dxr[v] <= 5).all()
    k = dxr * 21 + dy * 3

    w = np.zeros((NYT, KP, YT * W_IMG), np.float32)
    flat = (ty * KP + k) * (YT * W_IMG) + col
    for cc, vals in enumerate((a0, phi_x, phi_y)):
        np.add.at(w.ravel(), flat[v] + cc * (YT * W_IMG), vals[v])
    _cache["w"] = w
    return w


def _build_program():
    """Build the SPMD Bass program (same for every core)."""
    if "nc" in _cache:
        return _cache["nc"]
    import os
    g = _geometry()
    NB, Ns = g["NB"], g["Ns"]
    stripe_of, col_of, SPT, NGROUP = g["stripe_of"], g["col_of"], g["SPT"], g["NGROUP"]
    COLS = YT * W_IMG  # 2048 W columns per y-tile
    RCOLS = YTPC * NB * BC  # 1728 R columns
    warmup = int(os.environ.get("K_WARMUP", "0"))

    nc = bass.Bass()
    w_p = nc.declare_dram_parameter("w", [KP, YTPC * COLS], DT, isOutput=False)
    m12_p = nc.declare_dram_parameter("m12", [KP, 2 * RCOLS], DT, isOutput=False)
    out_p = nc.declare_dram_parameter(
        "out", [4, BC, NGROUP * PSUM_COLS], mybir.dt.float32, isOutput=True
    )

    with tile.TileContext(nc) as tc:
        with (
            tc.tile_pool(name="const", bufs=1) as const,
            tc.tile_pool(name="psum", bufs=4, space="PSUM") as psum,
            tc.tile_pool(name="wpsum", bufs=1, space="PSUM") as wpsum,
            tc.tile_pool(name="stage", bufs=1) as stagep,
        ):
            # all runtime tables in one DMA (SWDGE ring), W in one DMA (SP ring):
            # per-partition-contiguous DRAM layouts -> biggest descriptors
            m12_t = const.tile([KP, 2 * RCOLS], DT)
            nc.gpsimd.dma_start(out=m12_t[:], in_=m12_p[:])
            w_t = const.tile([KP, YTPC * COLS], DT)
            nc.sync.dma_start(out=w_t[:], in_=w_p[:])

            r_t = const.tile([KP, RCOLS], DT)

            if warmup:
                wu_a = const.tile([KP, 16], DT)
                wu_b = const.tile([KP, 128], DT)
                nc.vector.memset(wu_a[:], 0.0)
                nc.vector.memset(wu_b[:], 0.0)
                wu_p = wpsum.tile([128, 128], mybir.dt.float32)
                for _ in range(warmup):
                    nc.tensor.matmul(
                        wu_p[:16, :], wu_a[:], wu_b[:], start=True, stop=True,
                        tile_position=(0, 0),
                    )

            nc.vector.tensor_mul(r_t[:], m12_t[:, :RCOLS], m12_t[:, RCOLS:])

            stage_t = stagep.tile([128, NGROUP * PSUM_COLS], mybir.dt.float32)

            cur_group = -1
            pt = None
            for ty in range(YTPC):
                for bi in range(NB):
                    gs = ty * SPT + stripe_of[bi]  # global stripe
                    grp, lane = gs // 4, gs % 4
                    if grp != cur_group:
                        if pt is not None:
                            nc.any.tensor_copy(
                                stage_t[:, cur_group * PSUM_COLS:(cur_group + 1) * PSUM_COLS],
                                pt[:],
                            )
                        pt = psum.tile([128, PSUM_COLS], mybir.dt.float32, tag="pt")
                        cur_group = grp
                    xs = g["bands"][bi][0]
                    rcol = (ty * NB + bi) * BC
                    nc.tensor.matmul(
                        pt[32 * lane : 32 * lane + BC, col_of[bi] : col_of[bi] + Ns[bi]],
                        r_t[:, rcol : rcol + BC],
                        w_t[:, ty * COLS + YT * xs : ty * COLS + YT * xs + Ns[bi]],
                        start=True,
                        stop=True,
                        tile_position=(0, 32 * lane),
                    )
            nc.any.tensor_copy(
                stage_t[:, cur_group * PSUM_COLS:(cur_group + 1) * PSUM_COLS], pt[:]
            )
            for lane in range(4):
                eng = nc.sync if lane % 2 == 0 else nc.scalar
                eng.dma_start(
                    out=out_p[lane], in_=stage_t[32 * lane : 32 * lane + BC, :]
                )

    _legalize_sync_waits(nc)
    _cache["nc"] = nc
    return nc


def _host_inputs(cpoint_loc, alpha):
    """Per-core w / m12 rearrangements of the runtime inputs."""
    g = _geometry()
    I_map, c_of_k, NB = g["I_map"], g["c_of_k"], g["NB"]
    lx = cpoint_loc[..., 0]
    ly = cpoint_loc[..., 1]
    lval = np.stack([np.ones_like(lx), lx, ly])  # [3, B, K]

    # m1[ty, band, k, b] = lval[c(k), b, I[ty,band,k]] ; duplicated over ch
    m1b = lval[
        c_of_k[None, None, :, None],
        np.arange(B)[None, None, None, :],
        I_map[:, :, :, None],
    ]  # [NYT, NB, KDIM, B]
    m1 = np.repeat(m1b, 2, axis=3)  # [NYT, NB, KDIM, 16]
    m2 = alpha[
        np.arange(B)[None, None, None, :, None],
        I_map[:, :, :, None, None],
        np.arange(2)[None, None, None, None, :],
    ].reshape(NYT, NB, KDIM, BC)

    in_maps = []
    w = _build_w()
    npdt = mybir.dt.np(DT)
    RCOLS = YTPC * NB * BC
    for core in range(NCORES):
        sl = slice(core * YTPC, (core + 1) * YTPC)
        m12c = np.zeros((KP, 2 * RCOLS), npdt)
        m12c[:KDIM, :RCOLS] = m1[sl].transpose(2, 0, 1, 3).reshape(KDIM, -1)
        m12c[:KDIM, RCOLS:] = m2[sl].transpose(2, 0, 1, 3).reshape(KDIM, -1)
        wc = np.ascontiguousarray(
            w[sl].transpose(1, 0, 2).reshape(KP, -1)
        ).astype(npdt)
        in_maps.append({"w": wc, "m12": m12c})
    return in_maps


def _unshuffle(results):
    """Reassemble [B, 2, H, W] from the per-core staged outputs."""
    g = _geometry()
    bands, Ns = g["bands"], g["Ns"]
    stripe_of, col_of, SPT = g["stripe_of"], g["col_of"], g["SPT"]
    out = np.empty((B, 2, H, W_IMG), np.float32)
    for core in range(NCORES):
        res = results[core]["out"]  # [4, BC, NGROUP*512]
        for ty in range(YTPC):
            y0 = (core * YTPC + ty) * YT
            for bi, (xs, wd, _) in enumerate(bands):
                gs = ty * SPT + stripe_of[bi]
                grp, lane = gs // 4, gs % 4
                block = res[lane][:, grp * PSUM_COLS + col_of[bi] :
                                  grp * PSUM_COLS + col_of[bi] + Ns[bi]]
                out[:, :, y0 : y0 + YT, xs : xs + wd] = block.reshape(
                    B, 2, YT, wd
                )
    return out


def run(cpoint_loc, alpha, trace=False, trace_cores=None):
    nc = _build_program()
    in_maps = _host_inputs(np.asarray(cpoint_loc), np.asarray(alpha))
    res = run_bass_kernel_spmd(
        nc, in_maps, list(range(NCORES)), trace=trace,
        trace_cores=trace_cores if trace_cores is not None else
        (list(range(NCORES)) if trace else None),
    )
    return _unshuffle(res.results), res


def kernel(cpoint_loc, alpha, select_index=None, phi_0=None, phi_x=None,
           phi_y=None, cpoints_0=None, **_ignored):
    out, _ = run(np.asarray(cpoint_loc), np.asarray(alpha))
    return out


# revision 9
# speedup vs baseline: 2.0437x; 1.1082x over previous
"""Trainium2 Bass kernel for the RadialBasisArbitraryLayerT problem.

Math: for each pixel p=(y,x) and batch b:
    flow[b,ch,p] = sum_m phi[b,p,m] * alpha[b, idx[p,m], ch]
    phi[b,p,m]   = phi_0[p,m] + (lx[b,i]-cx0[p,m])*phi_x[p,m]
                              + (ly[b,i]-cy0[p,m])*phi_y[p,m],  i = idx[p,m]

All neighbor indices of a pixel live in a 6x6 window of the 32x32 control
grid, so the constant-index gather is converted into small dense matmuls:

    flow[bc, p] = sum_k  W[p, k] * R[k, bc]        k = (dx, dy, c) = 6*7*3 = 126
    W[p,(dx,dy,0)] = phi_0 - cx0*phi_x - cy0*phi_y   (constant, host-built)
    W[p,(dx,dy,1)] = phi_x,  W[p,(dx,dy,2)] = phi_y
    R[(dx,dy,c), (b,ch)] = l_c[b,i] * alpha[b,i,ch], l = (1, lx, ly)

W is a compile-time constant streamed from DRAM; R is computed on-chip from
a tiny host-gathered rearrangement (m1*m2) of the runtime inputs.

Sharding: 32 image rows (4 y-tiles of 8 rows) per core, 8 cores.
Each y-tile x x-band is one matmul: lhsT=R_chunk [126,16], rhs=W_tile [126,N],
out=PSUM[16, N] packed 4 stripes high (tile_position col groups) so PSUM
eviction runs 128 partitions wide. Output staged in SBUF, DMA'd per stripe
lane, final (y,x) reassembly on host.
"""

import numpy as np

import concourse.bass as bass
import concourse.tile as tile
from concourse import mybir
from concourse.bass_utils import run_bass_kernel_spmd

# ---------------------------------------------------------------- geometry
H = 256
W_IMG = 256
CH = 32
CW = 32
C = 24.0
B = 8
K = CH * CW
NCORES = 8
YT = 8                 # rows per y-tile
NYT = H // YT          # 32 y-tiles
YTPC = NYT // NCORES   # 4 y-tiles per core
KDIM = 6 * 7 * 3       # (dx, dy, c) window rows = 126
KP = 128               # padded contraction dim (DMA ports + FWL want 128)
BC = B * 2             # 16 output channels (b, ch)
PSUM_COLS = 512

DT = mybir.dt.float16          # on-chip dtype for W / R
NPDT = np.float16

_cache = {}


def _legalize_sync_waits(nc, max_waits=1):
    """This walrus build rejects instructions with more than one sync wait.
    Hoist extra waits into single-wait NOPs inserted just before the
    instruction on the same (sequential) engine — semantics preserved."""
    import bass_rust

    uid = 0
    for fn in nc.m.functions:
        for blk in fn.blocks:
            insts = blk.instructions
            new = []
            changed = False
            for inst in insts:
                si = inst.sync_info
                if si is not None and len(si.on_wait) > max_waits:
                    waits = list(si.on_wait)
                    for w in waits[:-max_waits]:
                        nop = bass_rust.InstNoOp(
                            name=f"legal-wait-{uid}",
                            engine=inst.engine,
                            ins=[],
                            outs=[],
                            sync_info=bass_rust.SyncInfo(on_wait=[w], on_update=[]),
                        )
                        uid += 1
                        new.append(nop)
                    inst.sync_info = bass_rust.SyncInfo(
                        on_wait=waits[-max_waits:], on_update=si.on_update
                    )
                    changed = True
                new.append(inst)
            if changed:
                blk.instructions = new


def _build_buffers():
    """Recompute the constant neighbor structure (mirrors the reference)."""
    cy = np.linspace(0.0, H - 1, CH, dtype=np.float32)
    cx = np.linspace(0.0, W_IMG - 1, CW, dtype=np.float32)
    gy, gx = np.meshgrid(cy, cx, indexing="ij")
    cp = np.stack([gx, gy], axis=-1).reshape(-1, 2).astype(np.float32)
    iy, ix = np.meshgrid(
        np.arange(H, dtype=np.float32),
        np.arange(W_IMG, dtype=np.float32),
        indexing="ij",
    )
    img = np.stack([ix, iy], axis=-1)
    dist = (
        np.linalg.norm(img[:, :, None, :] - cp[None, None, :, :], axis=3).astype(
            np.float32
        )
        / np.float32(C)
    )
    idx = np.argsort(dist, axis=2, kind="stable")
    sd = np.take_along_axis(dist, idx, axis=2)
    M = int((dist < 1.0).sum(axis=2).max())
    sd = sd[..., :M]
    idx = idx[..., :M].astype(np.int32)
    mask = (sd < 1.0).astype(np.float32)
    scp = cp[idx]
    one_m = 1.0 - sd
    phi_0 = (one_m**4) * (4.0 * sd + 1.0) * mask
    phi_r = -4.0 * (one_m**3) * (4.0 * sd + 1.0) + 4.0 * (one_m**4)
    denom = sd * np.float32(C * C) + np.float32(1e-5)
    r_x = (scp[..., 0] - img[:, :, None, 0]) / denom
    r_y = (scp[..., 1] - img[:, :, None, 1]) / denom
    phi_x = (phi_r * r_x * mask).astype(np.float32)
    phi_y = (phi_r * r_y * mask).astype(np.float32)
    return idx, phi_0.astype(np.float32), phi_x, phi_y, scp.astype(np.float32), mask


def _geometry():
    """Static tiling metadata: x-bands, y-windows, stripe packing, index maps."""
    if "geom" in _cache:
        return _cache["geom"]

    idx, phi_0, phi_x, phi_y, scp, mask = _build_buffers()
    M = idx.shape[-1]
    bmask = mask > 0.5
    gyi = idx // CW
    gxi = idx % CW

    # x-bands: maximal runs of x sharing one 6-wide gx window
    gx_min = np.where(bmask, gxi, 999).min(axis=(0, 2))
    gx_max = np.where(bmask, gxi, -1).max(axis=(0, 2))
    gx0 = np.minimum(gx_min, CW - 6)
    assert (gx_max - gx0 <= 5).all() and (gx0 >= 0).all()
    bands = []  # (x_start, width, gx0)
    s = 0
    for x in range(1, W_IMG + 1):
        if x == W_IMG or gx0[x] != gx0[s]:
            bands.append((s, x - s, int(gx0[s])))
            s = x
    NB = len(bands)

    # y-tiles: 8 rows, 7-wide gy window
    gy_min_row = np.where(bmask, gyi, 999).min(axis=(1, 2))
    gy_max_row = np.where(bmask, gyi, -1).max(axis=(1, 2))
    sy7 = []
    for t in range(NYT):
        lo = int(gy_min_row[YT * t : YT * t + YT].min())
        hi = int(gy_max_row[YT * t : YT * t + YT].max())
        sy = min(lo, CH - 7)
        assert hi - sy <= 6 and sy >= 0
        sy7.append(sy)

    # matmul N per band and greedy PSUM stripe packing (identical per y-tile)
    Ns = [YT * w for (_, w, _) in bands]
    stripe_of = []
    col_of = []
    sid, cur = 0, 0
    for N in Ns:
        if cur + N > PSUM_COLS:
            sid += 1
            cur = 0
        stripe_of.append(sid)
        col_of.append(cur)
        cur += N
    SPT = sid + 1                       # stripes per y-tile
    NSTRIPE = YTPC * SPT                # global stripes per core
    NGROUP = (NSTRIPE + 3) // 4         # PSUM tiles (4 stripes each) per core

    # per-chunk control point index map: I[ty, band, k] (k = dx*21 + dy*3 + c)
    dxr = np.arange(KDIM) // 21
    dy = (np.arange(KDIM) % 21) // 3
    c_of_k = np.arange(KDIM) % 3
    I_map = np.empty((NYT, NB, KDIM), np.int64)
    for t in range(NYT):
        for bi, (_, _, g0) in enumerate(bands):
            I_map[t, bi] = (sy7[t] + dy) * CW + (g0 + dxr)
    assert I_map.min() >= 0 and I_map.max() < K

    geom = dict(
        bands=bands, NB=NB, sy7=sy7, Ns=Ns, stripe_of=stripe_of, col_of=col_of,
        SPT=SPT, NSTRIPE=NSTRIPE, NGROUP=NGROUP, I_map=I_map, c_of_k=c_of_k,
        idx=idx, mask=bmask, phi_0=phi_0, phi_x=phi_x, phi_y=phi_y, scp=scp, M=M,
    )
    _cache["geom"] = geom
    return geom


def _build_w():
    """Constant weights W[ytile, KDIM, 2048] from the phi buffers."""
    if "w" in _cache:
        return _cache["w"]
    g = _geometry()
    bands, sy7 = g["bands"], g["sy7"]
    idx, bmask = g["idx"], g["mask"]
    phi_x, phi_y = g["phi_x"], g["phi_y"]
    a0 = g["phi_0"] - g["scp"][..., 0] * phi_x - g["scp"][..., 1] * phi_y
    M = g["M"]

    band_of_x = np.empty(W_IMG, np.int64)
    xs_of_x = np.empty(W_IMG, np.int64)
    wd_of_x = np.empty(W_IMG, np.int64)
    for bi, (xs, wd, _) in enumerate(bands):
        band_of_x[xs : xs + wd] = bi
        xs_of_x[xs : xs + wd] = xs
        wd_of_x[xs : xs + wd] = wd
    g0_of_x = np.array([bands[bi][2] for bi in band_of_x])

    yy, xx, _mm = np.meshgrid(
        np.arange(H), np.arange(W_IMG), np.arange(M), indexing="ij"
    )
    ty = yy // YT
    yl = yy % YT
    col = xs_of_x[xx] * YT + yl * wd_of_x[xx] + (xx - xs_of_x[xx])
    gyi = idx // CW
    gxi = idx % CW
    dy = gyi - np.array(sy7)[ty]
    dxr = gxi - g0_of_x[xx]
    v = bmask
    assert (dy[v] >= 0).all() and (dy[v] <= 6).all()
    assert (dxr[v] >= 0).all() and (dxr[v] <= 5).all()
    k = dxr * 21 + dy * 3

    w = np.zeros((NYT, KP, YT * W_IMG), np.float32)
    flat = (ty * KP + k) * (YT * W_IMG) + col
    for cc, vals in enumerate((a0, phi_x, phi_y)):
        np.add.at(w.ravel(), flat[v] + cc * (YT * W_IMG), vals[v])
    _cache["w"] = w
    return w


def _build_program():
    """Build the SPMD Bass program (same for every core)."""
    if "nc" in _cache:
        return _cache["nc"]
    import os
    g = _geometry()
    NB, Ns = g["NB"], g["Ns"]
    stripe_of, col_of, SPT, NGROUP = g["stripe_of"], g["col_of"], g["SPT"], g["NGROUP"]
    COLS = YT * W_IMG  # 2048 W columns per y-tile
    RCOLS = YTPC * NB * BC  # 1728 R columns
    warmup = int(os.environ.get("K_WARMUP", "0"))

    nc = bass.Bass()
    w_p = nc.declare_dram_parameter("w", [KP, YTPC * COLS], DT, isOutput=False)
    m12_p = nc.declare_dram_parameter("m12", [KP, 2 * RCOLS], DT, isOutput=False)
    out_p = nc.declare_dram_parameter(
        "out", [4, BC, NGROUP * PSUM_COLS], DT, isOutput=True
    )

    with tile.TileContext(nc) as tc:
        with (
            tc.tile_pool(name="const", bufs=1) as const,
            tc.tile_pool(name="psum", bufs=4, space="PSUM") as psum,
            tc.tile_pool(name="wpsum", bufs=1, space="PSUM") as wpsum,
            tc.tile_pool(name="stage", bufs=1) as stagep,
        ):
            # all runtime tables in one DMA (SWDGE ring), W in one DMA (SP ring):
            # per-partition-contiguous DRAM layouts -> biggest descriptors
            m12_t = const.tile([KP, 2 * RCOLS], DT)
            nc.sync.dma_start(out=m12_t[:], in_=m12_p[:])
            w_t = const.tile([KP, YTPC * COLS], DT)
            half = YTPC * COLS // 2
            nc.scalar.dma_start(out=w_t[:, :half], in_=w_p[:, :half])
            nc.sync.dma_start(out=w_t[:, half:], in_=w_p[:, half:])

            r_t = const.tile([KP, RCOLS], DT)

            if warmup:
                wu_a = const.tile([KP, 16], DT)
                wu_b = const.tile([KP, 128], DT)
                nc.vector.memset(wu_a[:], 0.0)
                nc.vector.memset(wu_b[:], 0.0)
                wu_p = wpsum.tile([128, 128], mybir.dt.float32)
                for _ in range(warmup):
                    nc.tensor.matmul(
                        wu_p[:16, :], wu_a[:], wu_b[:], start=True, stop=True,
                        tile_position=(0, 0),
                    )

            nc.vector.tensor_mul(r_t[:], m12_t[:, :RCOLS], m12_t[:, RCOLS:])

            stage_t = stagep.tile([128, NGROUP * PSUM_COLS], DT)

            cur_group = -1
            pt = None
            for ty in range(YTPC):
                for bi in range(NB):
                    gs = ty * SPT + stripe_of[bi]  # global stripe
                    grp, lane = gs // 4, gs % 4
                    if grp != cur_group:
                        if pt is not None:
                            nc.any.tensor_copy(
                                stage_t[:, cur_group * PSUM_COLS:(cur_group + 1) * PSUM_COLS],
                                pt[:],
                            )
                        pt = psum.tile([128, PSUM_COLS], mybir.dt.float32, tag="pt")
                        cur_group = grp
                    xs = g["bands"][bi][0]
                    rcol = (ty * NB + bi) * BC
                    nc.tensor.matmul(
                        pt[32 * lane : 32 * lane + BC, col_of[bi] : col_of[bi] + Ns[bi]],
                        r_t[:, rcol : rcol + BC],
                        w_t[:, ty * COLS + YT * xs : ty * COLS + YT * xs + Ns[bi]],
                        start=True,
                        stop=True,
                        tile_position=(0, 32 * lane),
                    )
            nc.any.tensor_copy(
                stage_t[:, cur_group * PSUM_COLS:(cur_group + 1) * PSUM_COLS], pt[:]
            )
            for lane in range(4):
                eng = nc.sync if lane % 2 == 0 else nc.scalar
                eng.dma_start(
                    out=out_p[lane], in_=stage_t[32 * lane : 32 * lane + BC, :]
                )

    _legalize_sync_waits(nc)
    _cache["nc"] = nc
    return nc


def _host_inputs(cpoint_loc, alpha):
    """Per-core w / m12 rearrangements of the runtime inputs."""
    g = _geometry()
    I_map, c_of_k, NB = g["I_map"], g["c_of_k"], g["NB"]
    lx = cpoint_loc[..., 0]
    ly = cpoint_loc[..., 1]
    lval = np.stack([np.ones_like(lx), lx, ly])  # [3, B, K]

    # m1[ty, band, k, b] = lval[c(k), b, I[ty,band,k]] ; duplicated over ch
    m1b = lval[
        c_of_k[None, None, :, None],
        np.arange(B)[None, None, None, :],
        I_map[:, :, :, None],
    ]  # [NYT, NB, KDIM, B]
    m1 = np.repeat(m1b, 2, axis=3)  # [NYT, NB, KDIM, 16]
    m2 = alpha[
        np.arange(B)[None, None, None, :, None],
        I_map[:, :, :, None, None],
        np.arange(2)[None, None, None, None, :],
    ].reshape(NYT, NB, KDIM, BC)

    in_maps = []
    w = _build_w()
    npdt = mybir.dt.np(DT)
    RCOLS = YTPC * NB * BC
    for core in range(NCORES):
        sl = slice(core * YTPC, (core + 1) * YTPC)
        m12c = np.zeros((KP, 2 * RCOLS), npdt)
        m12c[:KDIM, :RCOLS] = m1[sl].transpose(2, 0, 1, 3).reshape(KDIM, -1)
        m12c[:KDIM, RCOLS:] = m2[sl].transpose(2, 0, 1, 3).reshape(KDIM, -1)
        wc = np.ascontiguousarray(
            w[sl].transpose(1, 0, 2).reshape(KP, -1)
        ).astype(npdt)
        in_maps.append({"w": wc, "m12": m12c})
    return in_maps


def _unshuffle(results):
    """Reassemble [B, 2, H, W] from the per-core staged outputs."""
    g = _geometry()
    bands, Ns = g["bands"], g["Ns"]
    stripe_of, col_of, SPT = g["stripe_of"], g["col_of"], g["SPT"]
    out = np.empty((B, 2, H, W_IMG), np.float32)
    for core in range(NCORES):
        res = results[core]["out"]  # [4, BC, NGROUP*512]
        for ty in range(YTPC):
            y0 = (core * YTPC + ty) * YT
            for bi, (xs, wd, _) in enumerate(bands):
                gs = ty * SPT + stripe_of[bi]
                grp, lane = gs // 4, gs % 4
                block = res[lane][:, grp * PSUM_COLS + col_of[bi] :
                                  grp * PSUM_COLS + col_of[bi] + Ns[bi]]
                out[:, :, y0 : y0 + YT, xs : xs + wd] = block.reshape(
                    B, 2, YT, wd
                )
    return out


def run(cpoint_loc, alpha, trace=False, trace_cores=None):
    nc = _build_program()
    in_maps = _host_inputs(np.asarray(cpoint_loc), np.asarray(alpha))
    res = run_bass_kernel_spmd(
        nc, in_maps, list(range(NCORES)), trace=trace,
        trace_cores=trace_cores if trace_cores is not None else
        (list(range(NCORES)) if trace else None),
    )
    return _unshuffle(res.results), res


def kernel(cpoint_loc, alpha, select_index=None, phi_0=None, phi_x=None,
           phi_y=None, cpoints_0=None, **_ignored):
    out, _ = run(np.asarray(cpoint_loc), np.asarray(alpha))
    return out


# revision 11
# speedup vs baseline: 2.2171x; 1.0849x over previous
"""Trainium2 Bass kernel for the RadialBasisArbitraryLayerT problem.

Math: for each pixel p=(y,x) and batch b:
    flow[b,ch,p] = sum_m phi[b,p,m] * alpha[b, idx[p,m], ch]
    phi[b,p,m]   = phi_0[p,m] + (lx[b,i]-cx0[p,m])*phi_x[p,m]
                              + (ly[b,i]-cy0[p,m])*phi_y[p,m],  i = idx[p,m]

All neighbor indices of a pixel live in a 6x6 window of the 32x32 control
grid, so the constant-index gather is converted into small dense matmuls:

    flow[bc, p] = sum_k  W[p, k] * R[k, bc]        k = (dx, dy, c) = 6*7*3 = 126
    W[p,(dx,dy,0)] = phi_0 - cx0*phi_x - cy0*phi_y   (constant, host-built)
    W[p,(dx,dy,1)] = phi_x,  W[p,(dx,dy,2)] = phi_y
    R[(dx,dy,c), (b,ch)] = l_c[b,i] * alpha[b,i,ch], l = (1, lx, ly)

W is a compile-time constant streamed from DRAM; R is computed on-chip from
a tiny host-gathered rearrangement (m1*m2) of the runtime inputs.

Sharding: 32 image rows (4 y-tiles of 8 rows) per core, 8 cores.
Each y-tile x x-band is one matmul: lhsT=R_chunk [126,16], rhs=W_tile [126,N],
out=PSUM[16, N] packed 4 stripes high (tile_position col groups) so PSUM
eviction runs 128 partitions wide. Output staged in SBUF, DMA'd per stripe
lane, final (y,x) reassembly on host.
"""

import numpy as np

import concourse.bass as bass
import concourse.tile as tile
from concourse import mybir
from concourse.bass_utils import run_bass_kernel_spmd

# ---------------------------------------------------------------- geometry
H = 256
W_IMG = 256
CH = 32
CW = 32
C = 24.0
B = 8
K = CH * CW
NCORES = 8
YT = 8                 # rows per y-tile
NYT = H // YT          # 32 y-tiles
YTPC = NYT // NCORES   # 4 y-tiles per core
KDIM = 6 * 7 * 3       # (dx, dy, c) window rows = 126
KP = 128               # padded contraction dim (DMA ports + FWL want 128)
BC = B * 2             # 16 output channels (b, ch)
PSUM_COLS = 512

DT = mybir.dt.float16          # on-chip dtype for W / R
NPDT = np.float16

_cache = {}


def _slim_tile_exit():
    if _cache.get("slim_exit"):
        return
    from concourse.vector_clock import ScopedClock

    def _drain_and_barrier(self, tick_clock, wait_clock):
        nc = self.nc
        drain_inst = nc.sync.drain()
        wait_clock.add_sem_waits(
            drain_inst.ins, ScopedClock({None: tick_clock.global_clock})
        )
        nc.all_engine_barrier()
        assert self.sems is not None
        popped = nc._tile_sem_poison_stack.pop()
        assert popped is self._sem_poison
        nc.clear_and_free_semaphores(list(self.sems.allocated().values()))

    tile.TileContext._drain_and_barrier = _drain_and_barrier
    _cache["slim_exit"] = True


def _legalize_sync_waits(nc, max_waits=1):
    """This walrus build rejects instructions with more than one sync wait.
    Hoist extra waits into single-wait NOPs inserted just before the
    instruction on the same (sequential) engine — semantics preserved."""
    import bass_rust

    uid = 0
    for fn in nc.m.functions:
        for blk in fn.blocks:
            insts = blk.instructions
            new = []
            changed = False
            for inst in insts:
                si = inst.sync_info
                if si is not None and len(si.on_wait) > max_waits:
                    waits = list(si.on_wait)
                    for w in waits[:-max_waits]:
                        nop = bass_rust.InstNoOp(
                            name=f"legal-wait-{uid}",
                            engine=inst.engine,
                            ins=[],
                            outs=[],
                            sync_info=bass_rust.SyncInfo(on_wait=[w], on_update=[]),
                        )
                        uid += 1
                        new.append(nop)
                    inst.sync_info = bass_rust.SyncInfo(
                        on_wait=waits[-max_waits:], on_update=si.on_update
                    )
                    changed = True
                new.append(inst)
            if changed:
                blk.instructions = new


def _build_buffers():
    """Recompute the constant neighbor structure (mirrors the reference)."""
    cy = np.linspace(0.0, H - 1, CH, dtype=np.float32)
    cx = np.linspace(0.0, W_IMG - 1, CW, dtype=np.float32)
    gy, gx = np.meshgrid(cy, cx, indexing="ij")
    cp = np.stack([gx, gy], axis=-1).reshape(-1, 2).astype(np.float32)
    iy, ix = np.meshgrid(
        np.arange(H, dtype=np.float32),
        np.arange(W_IMG, dtype=np.float32),
        indexing="ij",
    )
    img = np.stack([ix, iy], axis=-1)
    dist = (
        np.linalg.norm(img[:, :, None, :] - cp[None, None, :, :], axis=3).astype(
            np.float32
        )
        / np.float32(C)
    )
    idx = np.argsort(dist, axis=2, kind="stable")
    sd = np.take_along_axis(dist, idx, axis=2)
    M = int((dist < 1.0).sum(axis=2).max())
    sd = sd[..., :M]
    idx = idx[..., :M].astype(np.int32)
    mask = (sd < 1.0).astype(np.float32)
    scp = cp[idx]
    one_m = 1.0 - sd
    phi_0 = (one_m**4) * (4.0 * sd + 1.0) * mask
    phi_r = -4.0 * (one_m**3) * (4.0 * sd + 1.0) + 4.0 * (one_m**4)
    denom = sd * np.float32(C * C) + np.float32(1e-5)
    r_x = (scp[..., 0] - img[:, :, None, 0]) / denom
    r_y = (scp[..., 1] - img[:, :, None, 1]) / denom
    phi_x = (phi_r * r_x * mask).astype(np.float32)
    phi_y = (phi_r * r_y * mask).astype(np.float32)
    return idx, phi_0.astype(np.float32), phi_x, phi_y, scp.astype(np.float32), mask


def _geometry():
    """Static tiling metadata: x-bands, y-windows, stripe packing, index maps."""
    if "geom" in _cache:
        return _cache["geom"]

    idx, phi_0, phi_x, phi_y, scp, mask = _build_buffers()
    M = idx.shape[-1]
    bmask = mask > 0.5
    gyi = idx // CW
    gxi = idx % CW

    # x-bands: maximal runs of x sharing one 6-wide gx window
    gx_min = np.where(bmask, gxi, 999).min(axis=(0, 2))
    gx_max = np.where(bmask, gxi, -1).max(axis=(0, 2))
    gx0 = np.minimum(gx_min, CW - 6)
    assert (gx_max - gx0 <= 5).all() and (gx0 >= 0).all()
    bands = []  # (x_start, width, gx0)
    s = 0
    for x in range(1, W_IMG + 1):
        if x == W_IMG or gx0[x] != gx0[s]:
            bands.append((s, x - s, int(gx0[s])))
            s = x
    NB = len(bands)

    # y-tiles: 8 rows, 7-wide gy window
    gy_min_row = np.where(bmask, gyi, 999).min(axis=(1, 2))
    gy_max_row = np.where(bmask, gyi, -1).max(axis=(1, 2))
    sy7 = []
    for t in range(NYT):
        lo = int(gy_min_row[YT * t : YT * t + YT].min())
        hi = int(gy_max_row[YT * t : YT * t + YT].max())
        sy = min(lo, CH - 7)
        assert hi - sy <= 6 and sy >= 0
        sy7.append(sy)

    # matmul N per band and greedy PSUM stripe packing (identical per y-tile)
    Ns = [YT * w for (_, w, _) in bands]
    stripe_of = []
    col_of = []
    sid, cur = 0, 0
    for N in Ns:
        if cur + N > PSUM_COLS:
            sid += 1
            cur = 0
        stripe_of.append(sid)
        col_of.append(cur)
        cur += N
    SPT = sid + 1                       # stripes per y-tile
    NSTRIPE = YTPC * SPT                # global stripes per core
    NGROUP = (NSTRIPE + 3) // 4         # PSUM tiles (4 stripes each) per core

    # per-chunk control point index map: I[ty, band, k] (k = dx*21 + dy*3 + c)
    dxr = np.arange(KDIM) // 21
    dy = (np.arange(KDIM) % 21) // 3
    c_of_k = np.arange(KDIM) % 3
    I_map = np.empty((NYT, NB, KDIM), np.int64)
    for t in range(NYT):
        for bi, (_, _, g0) in enumerate(bands):
            I_map[t, bi] = (sy7[t] + dy) * CW + (g0 + dxr)
    assert I_map.min() >= 0 and I_map.max() < K

    geom = dict(
        bands=bands, NB=NB, sy7=sy7, Ns=Ns, stripe_of=stripe_of, col_of=col_of,
        SPT=SPT, NSTRIPE=NSTRIPE, NGROUP=NGROUP, I_map=I_map, c_of_k=c_of_k,
        idx=idx, mask=bmask, phi_0=phi_0, phi_x=phi_x, phi_y=phi_y, scp=scp, M=M,
    )
    _cache["geom"] = geom
    return geom


def _build_w():
    """Constant weights W[ytile, KDIM, 2048] from the phi buffers."""
    if "w" in _cache:
        return _cache["w"]
    g = _geometry()
    bands, sy7 = g["bands"], g["sy7"]
    idx, bmask = g["idx"], g["mask"]
    phi_x, phi_y = g["phi_x"], g["phi_y"]
    a0 = g["phi_0"] - g["scp"][..., 0] * phi_x - g["scp"][..., 1] * phi_y
    M = g["M"]

    band_of_x = np.empty(W_IMG, np.int64)
    xs_of_x = np.empty(W_IMG, np.int64)
    wd_of_x = np.empty(W_IMG, np.int64)
    for bi, (xs, wd, _) in enumerate(bands):
        band_of_x[xs : xs + wd] = bi
        xs_of_x[xs : xs + wd] = xs
        wd_of_x[xs : xs + wd] = wd
    g0_of_x = np.array([bands[bi][2] for bi in band_of_x])

    yy, xx, _mm = np.meshgrid(
        np.arange(H), np.arange(W_IMG), np.arange(M), indexing="ij"
    )
    ty = yy // YT
    yl = yy % YT
    col = xs_of_x[xx] * YT + yl * wd_of_x[xx] + (xx - xs_of_x[xx])
    gyi = idx // CW
    gxi = idx % CW
    dy = gyi - np.array(sy7)[ty]
    dxr = gxi - g0_of_x[xx]
    v = bmask
    assert (dy[v] >= 0).all() and (dy[v] <= 6).all()
    assert (dxr[v] >= 0).all() and (dxr[v] <= 5).all()
    k = dxr * 21 + dy * 3

    w = np.zeros((NYT, KP, YT * W_IMG), np.float32)
    flat = (ty * KP + k) * (YT * W_IMG) + col
    for cc, vals in enumerate((a0, phi_x, phi_y)):
        np.add.at(w.ravel(), flat[v] + cc * (YT * W_IMG), vals[v])
    _cache["w"] = w
    return w


def _build_program():
    """Build the SPMD Bass program (same for every core)."""
    if "nc" in _cache:
        return _cache["nc"]
    import os
    g = _geometry()
    NB, Ns = g["NB"], g["Ns"]
    stripe_of, col_of, SPT, NGROUP = g["stripe_of"], g["col_of"], g["SPT"], g["NGROUP"]
    COLS = YT * W_IMG  # 2048 W columns per y-tile
    RCOLS = YTPC * NB * BC  # 1728 R columns
    warmup = int(os.environ.get("K_WARMUP", "0"))

    _slim_tile_exit()
    nc = bass.Bass()
    w_p = nc.declare_dram_parameter("w", [KP, YTPC * COLS], DT, isOutput=False)
    m12_p = nc.declare_dram_parameter("m12", [KP, 2 * RCOLS], DT, isOutput=False)
    out_p = nc.declare_dram_parameter(
        "out", [4, BC, NGROUP * PSUM_COLS], DT, isOutput=True
    )

    with tile.TileContext(nc) as tc:
        with (
            tc.tile_pool(name="const", bufs=1) as const,
            tc.tile_pool(name="psum", bufs=4, space="PSUM") as psum,
            tc.tile_pool(name="wpsum", bufs=1, space="PSUM") as wpsum,
            tc.tile_pool(name="stage", bufs=1) as stagep,
        ):
            # all runtime tables in one DMA (SWDGE ring), W in one DMA (SP ring):
            # per-partition-contiguous DRAM layouts -> biggest descriptors
            m12_t = const.tile([KP, 2 * RCOLS], DT)
            nc.sync.dma_start(out=m12_t[:], in_=m12_p[:])
            w_t = const.tile([KP, YTPC * COLS], DT)
            w_engines = [nc.scalar, nc.gpsimd, nc.scalar, nc.sync]
            for ty in range(YTPC):
                w_engines[ty].dma_start(
                    out=w_t[:, ty * COLS : (ty + 1) * COLS],
                    in_=w_p[:, ty * COLS : (ty + 1) * COLS],
                )

            r_t = const.tile([KP, RCOLS], DT)

            if warmup:
                wu_a = const.tile([KP, 16], DT)
                wu_b = const.tile([KP, 128], DT)
                nc.vector.memset(wu_a[:], 0.0)
                nc.vector.memset(wu_b[:], 0.0)
                wu_p = wpsum.tile([128, 128], mybir.dt.float32)
                for _ in range(warmup):
                    nc.tensor.matmul(
                        wu_p[:16, :], wu_a[:], wu_b[:], start=True, stop=True,
                        tile_position=(0, 0),
                    )

            nc.vector.tensor_mul(r_t[:], m12_t[:, :RCOLS], m12_t[:, RCOLS:])

            stage_t = stagep.tile([128, NGROUP * PSUM_COLS], DT)

            cur_group = -1
            pt = None
            for ty in range(YTPC):
                for bi in range(NB):
                    gs = ty * SPT + stripe_of[bi]  # global stripe
                    grp, lane = gs // 4, gs % 4
                    if grp != cur_group:
                        if pt is not None:
                            nc.any.tensor_copy(
                                stage_t[:, cur_group * PSUM_COLS:(cur_group + 1) * PSUM_COLS],
                                pt[:],
                            )
                        pt = psum.tile([128, PSUM_COLS], mybir.dt.float32, tag="pt")
                        cur_group = grp
                    xs = g["bands"][bi][0]
                    rcol = (ty * NB + bi) * BC
                    nc.tensor.matmul(
                        pt[32 * lane : 32 * lane + BC, col_of[bi] : col_of[bi] + Ns[bi]],
                        r_t[:, rcol : rcol + BC],
                        w_t[:, ty * COLS + YT * xs : ty * COLS + YT * xs + Ns[bi]],
                        start=True,
                        stop=True,
                        tile_position=(0, 32 * lane),
                    )
            nc.any.tensor_copy(
                stage_t[:, cur_group * PSUM_COLS:(cur_group + 1) * PSUM_COLS], pt[:]
            )
            for lane in range(4):
                eng = nc.sync if lane % 2 == 0 else nc.scalar
                eng.dma_start(
                    out=out_p[lane], in_=stage_t[32 * lane : 32 * lane + BC, :]
                )

    _legalize_sync_waits(nc)
    _cache["nc"] = nc
    return nc


def _host_inputs(cpoint_loc, alpha):
    """Per-core w / m12 rearrangements of the runtime inputs."""
    g = _geometry()
    I_map, c_of_k, NB = g["I_map"], g["c_of_k"], g["NB"]
    lx = cpoint_loc[..., 0]
    ly = cpoint_loc[..., 1]
    lval = np.stack([np.ones_like(lx), lx, ly])  # [3, B, K]

    # m1[ty, band, k, b] = lval[c(k), b, I[ty,band,k]] ; duplicated over ch
    m1b = lval[
        c_of_k[None, None, :, None],
        np.arange(B)[None, None, None, :],
        I_map[:, :, :, None],
    ]  # [NYT, NB, KDIM, B]
    m1 = np.repeat(m1b, 2, axis=3)  # [NYT, NB, KDIM, 16]
    m2 = alpha[
        np.arange(B)[None, None, None, :, None],
        I_map[:, :, :, None, None],
        np.arange(2)[None, None, None, None, :],
    ].reshape(NYT, NB, KDIM, BC)

    in_maps = []
    w = _build_w()
    npdt = mybir.dt.np(DT)
    RCOLS = YTPC * NB * BC
    for core in range(NCORES):
        sl = slice(core * YTPC, (core + 1) * YTPC)
        m12c = np.zeros((KP, 2 * RCOLS), npdt)
        m12c[:KDIM, :RCOLS] = m1[sl].transpose(2, 0, 1, 3).reshape(KDIM, -1)
        m12c[:KDIM, RCOLS:] = m2[sl].transpose(2, 0, 1, 3).reshape(KDIM, -1)
        wc = np.ascontiguousarray(
            w[sl].transpose(1, 0, 2).reshape(KP, -1)
        ).astype(npdt)
        in_maps.append({"w": wc, "m12": m12c})
    return in_maps


def _unshuffle(results):
    """Reassemble [B, 2, H, W] from the per-core staged outputs."""
    g = _geometry()
    bands, Ns = g["bands"], g["Ns"]
    stripe_of, col_of, SPT = g["stripe_of"], g["col_of"], g["SPT"]
    out = np.empty((B, 2, H, W_IMG), np.float32)
    for core in range(NCORES):
        res = results[core]["out"]  # [4, BC, NGROUP*512]
        for ty in range(YTPC):
            y0 = (core * YTPC + ty) * YT
            for bi, (xs, wd, _) in enumerate(bands):
                gs = ty * SPT + stripe_of[bi]
                grp, lane = gs // 4, gs % 4
                block = res[lane][:, grp * PSUM_COLS + col_of[bi] :
                                  grp * PSUM_COLS + col_of[bi] + Ns[bi]]
                out[:, :, y0 : y0 + YT, xs : xs + wd] = block.reshape(
                    B, 2, YT, wd
                )
    return out


def run(cpoint_loc, alpha, trace=False, trace_cores=None):
    nc = _build_program()
    in_maps = _host_inputs(np.asarray(cpoint_loc), np.asarray(alpha))
    res = run_bass_kernel_spmd(
        nc, in_maps, list(range(NCORES)), trace=trace,
        trace_cores=trace_cores if trace_cores is not None else
        (list(range(NCORES)) if trace else None),
    )
    return _unshuffle(res.results), res


def kernel(cpoint_loc, alpha, select_index=None, phi_0=None, phi_x=None,
           phi_y=None, cpoints_0=None, **_ignored):
    out, _ = run(np.asarray(cpoint_loc), np.asarray(alpha))
    return out


# revision 12
# speedup vs baseline: 2.2721x; 1.0248x over previous
"""Trainium2 Bass kernel for the RadialBasisArbitraryLayerT problem.

Math: for each pixel p=(y,x) and batch b:
    flow[b,ch,p] = sum_m phi[b,p,m] * alpha[b, idx[p,m], ch]
    phi[b,p,m]   = phi_0[p,m] + (lx[b,i]-cx0[p,m])*phi_x[p,m]
                              + (ly[b,i]-cy0[p,m])*phi_y[p,m],  i = idx[p,m]

All neighbor indices of a pixel live in a 6x6 window of the 32x32 control
grid, so the constant-index gather is converted into small dense matmuls:

    flow[bc, p] = sum_k  W[p, k] * R[k, bc]        k = (dx, dy, c) = 6*7*3 = 126
    W[p,(dx,dy,0)] = phi_0 - cx0*phi_x - cy0*phi_y   (constant, host-built)
    W[p,(dx,dy,1)] = phi_x,  W[p,(dx,dy,2)] = phi_y
    R[(dx,dy,c), (b,ch)] = l_c[b,i] * alpha[b,i,ch], l = (1, lx, ly)

W is a compile-time constant streamed from DRAM; R is computed on-chip from
a tiny host-gathered rearrangement (m1*m2) of the runtime inputs.

Sharding: 32 image rows (4 y-tiles of 8 rows) per core, 8 cores.
Each y-tile x x-band is one matmul: lhsT=R_chunk [126,16], rhs=W_tile [126,N],
out=PSUM[16, N] packed 4 stripes high (tile_position col groups) so PSUM
eviction runs 128 partitions wide. Output staged in SBUF, DMA'd per stripe
lane, final (y,x) reassembly on host.
"""

import numpy as np

import concourse.bass as bass
import concourse.tile as tile
from concourse import mybir
from concourse.bass_utils import run_bass_kernel_spmd

# ---------------------------------------------------------------- geometry
H = 256
W_IMG = 256
CH = 32
CW = 32
C = 24.0
B = 8
K = CH * CW
NCORES = 8
YT = 8                 # rows per y-tile
NYT = H // YT          # 32 y-tiles
YTPC = NYT // NCORES   # 4 y-tiles per core
KDIM = 6 * 7 * 3       # (dx, dy, c) window rows = 126
KP = 128               # padded contraction dim (DMA ports + FWL want 128)
BC = B * 2             # 16 output channels (b, ch)
PSUM_COLS = 512

DT = mybir.dt.float16          # on-chip dtype for W / R
NPDT = np.float16

_cache = {}


def _slim_tile_exit():
    if _cache.get("slim_exit"):
        return
    from concourse.vector_clock import ScopedClock

    def _drain_and_barrier(self, tick_clock, wait_clock):
        nc = self.nc
        drain_inst = nc.sync.drain()
        wait_clock.add_sem_waits(
            drain_inst.ins, ScopedClock({None: tick_clock.global_clock})
        )
        assert self.sems is not None
        popped = nc._tile_sem_poison_stack.pop()
        assert popped is self._sem_poison
        sems = list(self.sems.allocated().values())
        sem_nums = [s.num if hasattr(s, "num") else s for s in sems]
        nc._state.prepend_free_semaphores(sem_nums)
        for ps in nc._tile_sem_poison_stack:
            ps.update(sem_nums)

    tile.TileContext._drain_and_barrier = _drain_and_barrier
    _cache["slim_exit"] = True


def _legalize_sync_waits(nc, max_waits=1):
    """This walrus build rejects instructions with more than one sync wait.
    Hoist extra waits into single-wait NOPs inserted just before the
    instruction on the same (sequential) engine — semantics preserved."""
    import bass_rust

    uid = 0
    for fn in nc.m.functions:
        for blk in fn.blocks:
            insts = blk.instructions
            new = []
            changed = False
            for inst in insts:
                si = inst.sync_info
                if si is not None and len(si.on_wait) > max_waits:
                    waits = list(si.on_wait)
                    for w in waits[:-max_waits]:
                        nop = bass_rust.InstNoOp(
                            name=f"legal-wait-{uid}",
                            engine=inst.engine,
                            ins=[],
                            outs=[],
                            sync_info=bass_rust.SyncInfo(on_wait=[w], on_update=[]),
                        )
                        uid += 1
                        new.append(nop)
                    inst.sync_info = bass_rust.SyncInfo(
                        on_wait=waits[-max_waits:], on_update=si.on_update
                    )
                    changed = True
                new.append(inst)
            if changed:
                blk.instructions = new


def _build_buffers():
    """Recompute the constant neighbor structure (mirrors the reference)."""
    cy = np.linspace(0.0, H - 1, CH, dtype=np.float32)
    cx = np.linspace(0.0, W_IMG - 1, CW, dtype=np.float32)
    gy, gx = np.meshgrid(cy, cx, indexing="ij")
    cp = np.stack([gx, gy], axis=-1).reshape(-1, 2).astype(np.float32)
    iy, ix = np.meshgrid(
        np.arange(H, dtype=np.float32),
        np.arange(W_IMG, dtype=np.float32),
        indexing="ij",
    )
    img = np.stack([ix, iy], axis=-1)
    dist = (
        np.linalg.norm(img[:, :, None, :] - cp[None, None, :, :], axis=3).astype(
            np.float32
        )
        / np.float32(C)
    )
    idx = np.argsort(dist, axis=2, kind="stable")
    sd = np.take_along_axis(dist, idx, axis=2)
    M = int((dist < 1.0).sum(axis=2).max())
    sd = sd[..., :M]
    idx = idx[..., :M].astype(np.int32)
    mask = (sd < 1.0).astype(np.float32)
    scp = cp[idx]
    one_m = 1.0 - sd
    phi_0 = (one_m**4) * (4.0 * sd + 1.0) * mask
    phi_r = -4.0 * (one_m**3) * (4.0 * sd + 1.0) + 4.0 * (one_m**4)
    denom = sd * np.float32(C * C) + np.float32(1e-5)
    r_x = (scp[..., 0] - img[:, :, None, 0]) / denom
    r_y = (scp[..., 1] - img[:, :, None, 1]) / denom
    phi_x = (phi_r * r_x * mask).astype(np.float32)
    phi_y = (phi_r * r_y * mask).astype(np.float32)
    return idx, phi_0.astype(np.float32), phi_x, phi_y, scp.astype(np.float32), mask


def _geometry():
    """Static tiling metadata: x-bands, y-windows, stripe packing, index maps."""
    if "geom" in _cache:
        return _cache["geom"]

    idx, phi_0, phi_x, phi_y, scp, mask = _build_buffers()
    M = idx.shape[-1]
    bmask = mask > 0.5
    gyi = idx // CW
    gxi = idx % CW

    # x-bands: maximal runs of x sharing one 6-wide gx window
    gx_min = np.where(bmask, gxi, 999).min(axis=(0, 2))
    gx_max = np.where(bmask, gxi, -1).max(axis=(0, 2))
    gx0 = np.minimum(gx_min, CW - 6)
    assert (gx_max - gx0 <= 5).all() and (gx0 >= 0).all()
    bands = []  # (x_start, width, gx0)
    s = 0
    for x in range(1, W_IMG + 1):
        if x == W_IMG or gx0[x] != gx0[s]:
            bands.append((s, x - s, int(gx0[s])))
            s = x
    NB = len(bands)

    # y-tiles: 8 rows, 7-wide gy window
    gy_min_row = np.where(bmask, gyi, 999).min(axis=(1, 2))
    gy_max_row = np.where(bmask, gyi, -1).max(axis=(1, 2))
    sy7 = []
    for t in range(NYT):
        lo = int(gy_min_row[YT * t : YT * t + YT].min())
        hi = int(gy_max_row[YT * t : YT * t + YT].max())
        sy = min(lo, CH - 7)
        assert hi - sy <= 6 and sy >= 0
        sy7.append(sy)

    # matmul N per band and greedy PSUM stripe packing (identical per y-tile)
    Ns = [YT * w for (_, w, _) in bands]
    stripe_of = []
    col_of = []
    sid, cur = 0, 0
    for N in Ns:
        if cur + N > PSUM_COLS:
            sid += 1
            cur = 0
        stripe_of.append(sid)
        col_of.append(cur)
        cur += N
    SPT = sid + 1                       # stripes per y-tile
    NSTRIPE = YTPC * SPT                # global stripes per core
    NGROUP = (NSTRIPE + 3) // 4         # PSUM tiles (4 stripes each) per core

    # per-chunk control point index map: I[ty, band, k] (k = dx*21 + dy*3 + c)
    dxr = np.arange(KDIM) // 21
    dy = (np.arange(KDIM) % 21) // 3
    c_of_k = np.arange(KDIM) % 3
    I_map = np.empty((NYT, NB, KDIM), np.int64)
    for t in range(NYT):
        for bi, (_, _, g0) in enumerate(bands):
            I_map[t, bi] = (sy7[t] + dy) * CW + (g0 + dxr)
    assert I_map.min() >= 0 and I_map.max() < K

    geom = dict(
        bands=bands, NB=NB, sy7=sy7, Ns=Ns, stripe_of=stripe_of, col_of=col_of,
        SPT=SPT, NSTRIPE=NSTRIPE, NGROUP=NGROUP, I_map=I_map, c_of_k=c_of_k,
        idx=idx, mask=bmask, phi_0=phi_0, phi_x=phi_x, phi_y=phi_y, scp=scp, M=M,
    )
    _cache["geom"] = geom
    return geom


def _build_w():
    """Constant weights W[ytile, KDIM, 2048] from the phi buffers."""
    if "w" in _cache:
        return _cache["w"]
    g = _geometry()
    bands, sy7 = g["bands"], g["sy7"]
    idx, bmask = g["idx"], g["mask"]
    phi_x, phi_y = g["phi_x"], g["phi_y"]
    a0 = g["phi_0"] - g["scp"][..., 0] * phi_x - g["scp"][..., 1] * phi_y
    M = g["M"]

    band_of_x = np.empty(W_IMG, np.int64)
    xs_of_x = np.empty(W_IMG, np.int64)
    wd_of_x = np.empty(W_IMG, np.int64)
    for bi, (xs, wd, _) in enumerate(bands):
        band_of_x[xs : xs + wd] = bi
        xs_of_x[xs : xs + wd] = xs
        wd_of_x[xs : xs + wd] = wd
    g0_of_x = np.array([bands[bi][2] for bi in band_of_x])

    yy, xx, _mm = np.meshgrid(
        np.arange(H), np.arange(W_IMG), np.arange(M), indexing="ij"
    )
    ty = yy // YT
    yl = yy % YT
    col = xs_of_x[xx] * YT + yl * wd_of_x[xx] + (xx - xs_of_x[xx])
    gyi = idx // CW
    gxi = idx % CW
    dy = gyi - np.array(sy7)[ty]
    dxr = gxi - g0_of_x[xx]
    v = bmask
    assert (dy[v] >= 0).all() and (dy[v] <= 6).all()
    assert (dxr[v] >= 0).all() and (dxr[v] <= 5).all()
    k = dxr * 21 + dy * 3

    w = np.zeros((NYT, KP, YT * W_IMG), np.float32)
    flat = (ty * KP + k) * (YT * W_IMG) + col
    for cc, vals in enumerate((a0, phi_x, phi_y)):
        np.add.at(w.ravel(), flat[v] + cc * (YT * W_IMG), vals[v])
    _cache["w"] = w
    return w


def _build_program():
    """Build the SPMD Bass program (same for every core)."""
    if "nc" in _cache:
        return _cache["nc"]
    import os
    g = _geometry()
    NB, Ns = g["NB"], g["Ns"]
    stripe_of, col_of, SPT, NGROUP = g["stripe_of"], g["col_of"], g["SPT"], g["NGROUP"]
    COLS = YT * W_IMG  # 2048 W columns per y-tile
    RCOLS = YTPC * NB * BC  # 1728 R columns
    warmup = int(os.environ.get("K_WARMUP", "0"))

    _slim_tile_exit()
    nc = bass.Bass()
    w_p = nc.declare_dram_parameter("w", [KP, YTPC * COLS], DT, isOutput=False)
    m12_p = nc.declare_dram_parameter("m12", [KP, 2 * RCOLS], DT, isOutput=False)
    out_p = nc.declare_dram_parameter(
        "out", [4, BC, NGROUP * PSUM_COLS], DT, isOutput=True
    )

    with tile.TileContext(nc) as tc:
        with (
            tc.tile_pool(name="const", bufs=1) as const,
            tc.tile_pool(name="psum", bufs=4, space="PSUM") as psum,
            tc.tile_pool(name="wpsum", bufs=1, space="PSUM") as wpsum,
            tc.tile_pool(name="stage", bufs=1) as stagep,
        ):
            # all runtime tables in one DMA (SWDGE ring), W in one DMA (SP ring):
            # per-partition-contiguous DRAM layouts -> biggest descriptors
            m12_t = const.tile([KP, 2 * RCOLS], DT)
            nc.sync.dma_start(out=m12_t[:], in_=m12_p[:])
            w_t = const.tile([KP, YTPC * COLS], DT)
            w_engines = [nc.scalar, nc.gpsimd, nc.scalar, nc.sync]
            for ty in range(YTPC):
                w_engines[ty].dma_start(
                    out=w_t[:, ty * COLS : (ty + 1) * COLS],
                    in_=w_p[:, ty * COLS : (ty + 1) * COLS],
                )

            r_t = const.tile([KP, RCOLS], DT)

            if warmup:
                wu_a = const.tile([KP, 16], DT)
                wu_b = const.tile([KP, 128], DT)
                nc.vector.memset(wu_a[:], 0.0)
                nc.vector.memset(wu_b[:], 0.0)
                wu_p = wpsum.tile([128, 128], mybir.dt.float32)
                for _ in range(warmup):
                    nc.tensor.matmul(
                        wu_p[:16, :], wu_a[:], wu_b[:], start=True, stop=True,
                        tile_position=(0, 0),
                    )

            nc.vector.tensor_mul(r_t[:], m12_t[:, :RCOLS], m12_t[:, RCOLS:])

            stage_t = stagep.tile([128, NGROUP * PSUM_COLS], DT)

            cur_group = -1
            pt = None
            for ty in range(YTPC):
                for bi in range(NB):
                    gs = ty * SPT + stripe_of[bi]  # global stripe
                    grp, lane = gs // 4, gs % 4
                    if grp != cur_group:
                        if pt is not None:
                            nc.any.tensor_copy(
                                stage_t[:, cur_group * PSUM_COLS:(cur_group + 1) * PSUM_COLS],
                                pt[:],
                            )
                        pt = psum.tile([128, PSUM_COLS], mybir.dt.float32, tag="pt")
                        cur_group = grp
                    xs = g["bands"][bi][0]
                    rcol = (ty * NB + bi) * BC
                    nc.tensor.matmul(
                        pt[32 * lane : 32 * lane + BC, col_of[bi] : col_of[bi] + Ns[bi]],
                        r_t[:, rcol : rcol + BC],
                        w_t[:, ty * COLS + YT * xs : ty * COLS + YT * xs + Ns[bi]],
                        start=True,
                        stop=True,
                        tile_position=(0, 32 * lane),
                    )
            nc.any.tensor_copy(
                stage_t[:, cur_group * PSUM_COLS:(cur_group + 1) * PSUM_COLS], pt[:]
            )
            for lane in range(4):
                eng = nc.sync if lane % 2 == 0 else nc.scalar
                eng.dma_start(
                    out=out_p[lane], in_=stage_t[32 * lane : 32 * lane + BC, :]
                )

    _legalize_sync_waits(nc)
    _cache["nc"] = nc
    return nc


def _host_inputs(cpoint_loc, alpha):
    """Per-core w / m12 rearrangements of the runtime inputs."""
    g = _geometry()
    I_map, c_of_k, NB = g["I_map"], g["c_of_k"], g["NB"]
    lx = cpoint_loc[..., 0]
    ly = cpoint_loc[..., 1]
    lval = np.stack([np.ones_like(lx), lx, ly])  # [3, B, K]

    # m1[ty, band, k, b] = lval[c(k), b, I[ty,band,k]] ; duplicated over ch
    m1b = lval[
        c_of_k[None, None, :, None],
        np.arange(B)[None, None, None, :],
        I_map[:, :, :, None],
    ]  # [NYT, NB, KDIM, B]
    m1 = np.repeat(m1b, 2, axis=3)  # [NYT, NB, KDIM, 16]
    m2 = alpha[
        np.arange(B)[None, None, None, :, None],
        I_map[:, :, :, None, None],
        np.arange(2)[None, None, None, None, :],
    ].reshape(NYT, NB, KDIM, BC)

    in_maps = []
    w = _build_w()
    npdt = mybir.dt.np(DT)
    RCOLS = YTPC * NB * BC
    for core in range(NCORES):
        sl = slice(core * YTPC, (core + 1) * YTPC)
        m12c = np.zeros((KP, 2 * RCOLS), npdt)
        m12c[:KDIM, :RCOLS] = m1[sl].transpose(2, 0, 1, 3).reshape(KDIM, -1)
        m12c[:KDIM, RCOLS:] = m2[sl].transpose(2, 0, 1, 3).reshape(KDIM, -1)
        wc = np.ascontiguousarray(
            w[sl].transpose(1, 0, 2).reshape(KP, -1)
        ).astype(npdt)
        in_maps.append({"w": wc, "m12": m12c})
    return in_maps


def _unshuffle(results):
    """Reassemble [B, 2, H, W] from the per-core staged outputs."""
    g = _geometry()
    bands, Ns = g["bands"], g["Ns"]
    stripe_of, col_of, SPT = g["stripe_of"], g["col_of"], g["SPT"]
    out = np.empty((B, 2, H, W_IMG), np.float32)
    for core in range(NCORES):
        res = results[core]["out"]  # [4, BC, NGROUP*512]
        for ty in range(YTPC):
            y0 = (core * YTPC + ty) * YT
            for bi, (xs, wd, _) in enumerate(bands):
                gs = ty * SPT + stripe_of[bi]
                grp, lane = gs // 4, gs % 4
                block = res[lane][:, grp * PSUM_COLS + col_of[bi] :
                                  grp * PSUM_COLS + col_of[bi] + Ns[bi]]
                out[:, :, y0 : y0 + YT, xs : xs + wd] = block.reshape(
                    B, 2, YT, wd
                )
    return out


def run(cpoint_loc, alpha, trace=False, trace_cores=None):
    nc = _build_program()
    in_maps = _host_inputs(np.asarray(cpoint_loc), np.asarray(alpha))
    res = run_bass_kernel_spmd(
        nc, in_maps, list(range(NCORES)), trace=trace,
        trace_cores=trace_cores if trace_cores is not None else
        (list(range(NCORES)) if trace else None),
    )
    return _unshuffle(res.results), res


def kernel(cpoint_loc, alpha, select_index=None, phi_0=None, phi_x=None,
           phi_y=None, cpoints_0=None, **_ignored):
    out, _ = run(np.asarray(cpoint_loc), np.asarray(alpha))
    return out
